# revision 22
# baseline (speedup 1.0000x reference)
"""Trainium2 Bass kernel for GetMask (blur + brightness-match + motion mask +
erode/dilate), data-parallel over the batch across 8 NeuronCores.

Self-contained: builds the Bass/Tile program, shards the batch 2-per-core,
runs via run_bass_kernel_spmd, and reassembles full outputs.

Algorithm notes (math is restructured but equivalent to the reference):
  blur_s = 25 * avg_pool5x5(x)  (zero-padded separable sum, done as 5
           horizontally-shifted accumulating matmuls with a 5-diagonal band
           matrix on the tensor engine, bf16 in / fp32 psum out)
  stats:  factor = mean(r_blur)/mean(nr_blur) = S_rw / S_nr  where S_rw is a
          (3,4,5,...,5,4,3) separably-weighted raw sum and S_nr the plain sum
          of blurred nr; global min/max of blurred nr; all-reduced via a tiny
          AllGather across the 8 cores.
  mask  = any_c(|A*clip(f*nr_blur,0,1)+B - r_blur| > 0.3)
        = max_c |min(relu(f/25*S_nr),1) - (S_r/25 - B)/A| > 0.3/A
  erode = 11x11 min-pool (replicate border) = H min-tree + vertical 11-sum
          band matmul, count >= 10.5 (out-of-image rows forced to 1)
  dilate= 11x11 max-pool = H max-tree + vertical 11-sum matmul, count >= 0.5
Output channels are identical by construction; both output tensors are
written in full from the device.
"""

import os
import sys

if "/opt/trn_rl_repo" not in sys.path:
    sys.path.insert(0, "/opt/trn_rl_repo")

# debug bisection switches
SKIP_STATS = os.environ.get("KMASK_SKIP_STATS", "0") == "1"
SKIP_A = os.environ.get("KMASK_SKIP_A", "0") == "1"
NO_CC = os.environ.get("KMASK_NO_CC", "0") == "1"

import numpy as np
import ml_dtypes

import concourse.bass as bass
import concourse.mybir as mybir
import concourse.bacc as bacc
import concourse.tile as tile
from concourse.bass_utils import run_bass_kernel_spmd


def _install_axon_ntff_hook():
    """The agent image's antenv lacks axon_hooks; synthesize it so
    run_bass_kernel_spmd(trace=True) can capture NTFF profiles via the
    injected libaxon_pjrt.so (same ctypes ABI trn_boot uses)."""
    import types
    import ctypes
    import contextlib

    try:
        import antenv.axon_hooks  # noqa: F401
        return
    except ImportError:
        pass
    so_path = "/opt/axon/libaxon_pjrt.so"
    if not os.path.exists(so_path):
        return
    try:
        lib = ctypes.CDLL(so_path)
    except OSError:
        return
    if not hasattr(lib, "axon_start_nrt_profile"):
        return
    lib.axon_start_nrt_profile.argtypes = [
        ctypes.POINTER(ctypes.c_int64),
        ctypes.c_size_t,
    ]
    lib.axon_start_nrt_profile.restype = ctypes.c_int64
    lib.axon_stop_nrt_profile.argtypes = [ctypes.c_char_p]
    lib.axon_stop_nrt_profile.restype = ctypes.c_int64

    @contextlib.contextmanager
    def _hook(output_dir, device_ids):
        import jax

        jax.devices()
        if device_ids:
            ids = (ctypes.c_int64 * len(device_ids))(*device_ids)
            rc = lib.axon_start_nrt_profile(ids, len(device_ids))
        else:
            rc = lib.axon_start_nrt_profile(None, 0)
        if rc != 0:
            raise RuntimeError(f"axon_start_nrt_profile rc={rc}")
        try:
            yield
        finally:
            n = lib.axon_stop_nrt_profile(str(output_dir).encode())
            print(f"ntff profile: {n} file(s) -> {output_dir}", flush=True)

    mod = types.ModuleType("antenv.axon_hooks")
    mod.get_axon_ntff_profile_hook = lambda: _hook
    mod.set_axon_ntff_profile_hook = lambda h: None
    sys.modules["antenv.axon_hooks"] = mod


_install_axon_ntff_hook()

# artifact upload to the share bucket may be unavailable in this container;
# degrade to local-only trace processing
import concourse.bass_utils as _bu

_orig_upload = _bu.upload_artifacts


def _safe_upload(tmpdir):
    try:
        return _orig_upload(tmpdir)
    except Exception as e:  # noqa: BLE001
        print(f"(artifact upload skipped: {type(e).__name__})", flush=True)
        return tmpdir


_bu.upload_artifacts = _safe_upload

f32 = mybir.dt.float32
bf16 = mybir.dt.bfloat16
AF = mybir.ActivationFunctionType
OP = mybir.AluOpType

B, C, H, W = 16, 3, 1024, 1024
N_CORES = 8
B_LOC = B // N_CORES  # 2 images per core

KB = 5    # blur kernel
KM = 11   # erode/dilate kernel

# phase A (stats) tiling: nr stride 124 (halo 4), r stride 128 (no halo)
A_STRIDE = 124
A_BLOCKS = 9
# phase B tiling: output stride 104 (halo 24)
B_STRIDE = 104
B_BLOCKS = 10


def _edge_w(n):
    w = np.full(n, 5.0, np.float64)
    w[0] = 3.0
    w[1] = 4.0
    w[-1] = 3.0
    w[-2] = 4.0
    return w


def _band(k_rows, m_cols, width):
    b = np.zeros((k_rows, m_cols), np.float32)
    for m in range(m_cols):
        b[m : m + width, m] = 1.0
    return b


def make_const_inputs():
    wrow = np.zeros((128, 8), np.float64)
    we = _edge_w(H)
    for t in range(8):
        wrow[:, t] = we[128 * t : 128 * (t + 1)]
    wcol = _edge_w(W).astype(np.float32)[None, :]
    return {
        "band_blur": _band(128, 124, KB).astype(ml_dtypes.bfloat16),
        "band_erode": _band(128, 114, KM).astype(ml_dtypes.bfloat16),
        "band_dilate": _band(128, 104, KM).astype(ml_dtypes.bfloat16),
        "wrow": wrow.astype(ml_dtypes.bfloat16),
        "wcol": wcol.astype(np.float32),
        "ones128": np.ones((1, 128), np.float32),
        "ident": np.eye(128, dtype=np.float32),
    }



def _memset_rows(eng_memset, ap_of, lo, hi, val):
    """Memset partitions [lo, hi) in chunks legal for engine APs: each chunk
    starts at a multiple of 32 and spans at most 32 partitions. lo is rounded
    down to a quadrant boundary; the caller must overwrite the gap after."""
    s = (lo // 32) * 32
    while s < hi:
        e = min(s + 32, hi)
        eng_memset(ap_of(s, e), val)
        s = e


def build_body(nc, tc, n_cores, debug=False):
    nr_d = nc.dram_tensor("non_refer", [B_LOC, C, H, W], f32, kind="ExternalInput")
    rf_d = nc.dram_tensor("refer", [B_LOC, C, H, W], f32, kind="ExternalInput")
    bb_d = nc.dram_tensor("band_blur", [128, 124], bf16, kind="ExternalInput")
    be_d = nc.dram_tensor("band_erode", [128, 114], bf16, kind="ExternalInput")
    bd_d = nc.dram_tensor("band_dilate", [128, 104], bf16, kind="ExternalInput")
    wr_d = nc.dram_tensor("wrow", [128, 8], bf16, kind="ExternalInput")
    wc_d = nc.dram_tensor("wcol", [1, W], f32, kind="ExternalInput")
    on_d = nc.dram_tensor("ones128", [1, 128], f32, kind="ExternalInput")
    id_d = nc.dram_tensor("ident", [128, 128], f32, kind="ExternalInput")
    gh_d = nc.dram_tensor("ghost", [B_LOC, C, H, W], f32, kind="ExternalOutput")
    ng_d = nc.dram_tensor("nonghost", [B_LOC, C, H, W], f32, kind="ExternalOutput")

    nr_ap, rf_ap = nr_d.ap(), rf_d.ap()
    gh_ap, ng_ap = gh_d.ap(), ng_d.ap()
    dbg_ap = None
    if debug:
        dbg_d = nc.dram_tensor(
            "dbg_mask", [B_LOC, B_BLOCKS, 124, W], bf16, kind="ExternalOutput"
        )
        dbg_ap = dbg_d.ap()

    with tc.tile_pool(name="const", bufs=1) as cpool, \
         tc.tile_pool(name="acc", bufs=1) as accp:
        bb = cpool.tile([128, 124], bf16)
        nc.sync.dma_start(bb[:], bb_d.ap()[:])
        be = cpool.tile([128, 114], bf16)
        nc.sync.dma_start(be[:], be_d.ap()[:])
        bd = cpool.tile([128, 104], bf16)
        nc.sync.dma_start(bd[:], bd_d.ap()[:])
        wr = cpool.tile([128, 8], bf16)
        nc.sync.dma_start(wr[:], wr_d.ap()[:])
        wc = cpool.tile([1, W], f32)
        nc.sync.dma_start(wc[:], wc_d.ap()[:])
        ones = cpool.tile([1, 128], f32)
        nc.sync.dma_start(ones[:], on_d.ap()[:])
        ident = cpool.tile([128, 128], f32)
        nc.sync.dma_start(ident[:], id_d.ap()[:])

        cm10 = cpool.tile([128, 1], f32, tag="cm10")
        nc.vector.memset(cm10[:], -10.0)
        cp1 = cpool.tile([128, 1], f32, tag="cp1")
        nc.vector.memset(cp1[:], 1.0)
        acc_min = accp.tile([128, 1], f32, tag="amin")
        acc_max = accp.tile([128, 1], f32, tag="amax")
        acc_sum = accp.tile([128, 1], f32, tag="asum")
        nc.vector.memset(acc_min[:], 1e30)
        nc.vector.memset(acc_max[:], -1e30)
        nc.vector.memset(acc_sum[:], 0.0)
        bcast = accp.tile([128, 5], f32, tag="bcast")

        # ---------------- Phase A: stats ----------------
        with tc.tile_pool(name="araw", bufs=10) as araw, \
             tc.tile_pool(name="abf", bufs=8) as abf, \
             tc.tile_pool(name="ablur", bufs=4) as ablur, \
             tc.tile_pool(name="atiny", bufs=8) as atiny, \
             tc.tile_pool(name="apsum", bufs=2, space="PSUM") as apsum, \
             tc.tile_pool(name="rpsum", bufs=1, space="PSUM") as rpsum:

            psr0 = rpsum.tile([1, 512], f32, tag="psr0")
            psr1 = rpsum.tile([1, 512], f32, tag="psr1")

            for img in range(B_LOC if not SKIP_A else 0):
                for ch in range(C):
                    # --- r tiles: plain 128-stride, weighted raw sums on PE
                    for t in range(8):
                        rt = araw.tile([128, W], f32, tag="raw")
                        nc.sync.dma_start(
                            rt[:], rf_ap[img, ch, 128 * t : 128 * (t + 1), :]
                        )
                        rb = abf.tile([128, W + 4], bf16, tag="bf")
                        nc.vector.tensor_copy(rb[:, 2 : W + 2], rt[:])
                        last = img == B_LOC - 1 and ch == C - 1 and t == 7
                        first = img == 0 and ch == 0 and t == 0
                        nc.tensor.matmul(
                            psr0[:], lhsT=wr[:, t : t + 1], rhs=rb[:, 2:514],
                            start=first, stop=last,
                        )
                        nc.tensor.matmul(
                            psr1[:], lhsT=wr[:, t : t + 1],
                            rhs=rb[:, 514 : W + 2],
                            start=first, stop=last,
                        )
                    # --- nr tiles: 124-stride with halo, blur + min/max/sum
                    for t in range(A_BLOCKS):
                        g0 = A_STRIDE * t - 2
                        lo, hi = max(0, g0), min(H, g0 + 128)
                        p0, p1 = lo - g0, hi - g0
                        v = min(124, H - A_STRIDE * t)
                        nt = araw.tile([128, W], f32, tag="raw")
                        if p0 > 0:
                            nc.scalar.memzero(nt[0:p0, :])
                        if p1 < 128:
                            _memset_rows(
                                nc.vector.memset, lambda s, e: nt[s:e, :], p1, 128, 0.0
                            )
                        nc.sync.dma_start(nt[p0:p1, :], nr_ap[img, ch, lo:hi, :])
                        nb = abf.tile([128, W + 4], bf16, tag="bf")
                        nc.vector.memset(nb[:, 0:2], 0.0)
                        nc.vector.memset(nb[:, W + 2 : W + 4], 0.0)
                        nc.scalar.activation(nb[:, 2 : W + 2], nt[:], AF.Copy)
                        ps = apsum.tile([124, W], f32, tag="apsum")
                        for si, d in enumerate((-2, -1, 0, 1, 2)):
                            for c0 in (0, 512):
                                nc.tensor.matmul(
                                    ps[:, c0 : c0 + 512],
                                    lhsT=bb[:],
                                    rhs=nb[:, 2 + d + c0 : 2 + d + c0 + 512],
                                    start=si == 0,
                                    stop=si == 4,
                                )
                        blur = ablur.tile([124, W], bf16, tag="blur")
                        tsum = atiny.tile([128, 1], f32, tag="tsum")
                        nc.scalar.activation(
                            blur[:], ps[:], AF.Copy, accum_out=tsum[0:124, :]
                        )
                        tmin = atiny.tile([128, 1], f32, tag="tmin")
                        tmax = atiny.tile([128, 1], f32, tag="tmax")
                        nc.vector.tensor_reduce(
                            tmin[0:v, :], blur[0:v, :], axis=mybir.AxisListType.X,
                            op=OP.min,
                        )
                        nc.vector.tensor_reduce(
                            tmax[0:v, :], blur[0:v, :], axis=mybir.AxisListType.X,
                            op=OP.max,
                        )
                        nc.vector.tensor_tensor(
                            acc_min[0:v, :], acc_min[0:v, :], tmin[0:v, :], op=OP.min
                        )
                        nc.vector.tensor_tensor(
                            acc_max[0:v, :], acc_max[0:v, :], tmax[0:v, :], op=OP.max
                        )
                        nc.vector.tensor_tensor(
                            acc_sum[0:v, :], acc_sum[0:v, :], tsum[0:v, :], op=OP.add
                        )

            # ---- fold local stats to scalars (psr still in scope) ----
            if SKIP_STATS:
                # fixed plausible constants: f=1 -> f25=0.04; A=1 -> inv25A=0.04;
                # B=0 -> negBoverA=0; thr=0.3; negthr=-0.3
                nc.vector.memset(bcast[:, 0:1], 0.04)
                nc.vector.memset(bcast[:, 1:2], 0.04)
                nc.vector.memset(bcast[:, 2:3], 0.0)
                nc.vector.memset(bcast[:, 3:4], 0.3)
                nc.vector.memset(bcast[:, 4:5], -0.3)
            if (not SKIP_STATS) and True:
             with tc.tile_pool(name="stat", bufs=1) as sp, \
                 tc.tile_pool(name="ccdram", bufs=1, space="DRAM") as dram:
                colw = sp.tile([1, 512], f32, tag="colw")
                colw2 = sp.tile([1, 512], f32, tag="colw2")
                racc = sp.tile([1, 1], f32, tag="racc")
                racc2 = sp.tile([1, 1], f32, tag="racc2")
                rtot = sp.tile([1, 1], f32, tag="rtot")
                nc.vector.tensor_tensor(colw[:], psr0[:], wc[:, 0:512], op=OP.mult)
                nc.vector.tensor_tensor(colw2[:], psr1[:], wc[:, 512:W], op=OP.mult)
                nc.vector.tensor_reduce(
                    racc[:], colw[:], axis=mybir.AxisListType.X, op=OP.add
                )
                nc.vector.tensor_reduce(
                    racc2[:], colw2[:], axis=mybir.AxisListType.X, op=OP.add
                )
                nc.vector.tensor_tensor(rtot[:], racc[:], racc2[:], op=OP.add)
                # pack local stats [sum, -min, max, rtot] as [128,4], transpose
                # on the PE, and ship [4,128] through the AllGather
                negmin = sp.tile([128, 1], f32, tag="negmin")
                nc.vector.tensor_scalar(negmin[:], acc_min[:], -1.0, None, OP.mult)
                packed = sp.tile([128, 4], f32, tag="packed")
                nc.vector.memset(packed[:, 3:4], 0.0)
                nc.vector.tensor_copy(packed[:, 0:1], acc_sum[:])
                nc.vector.tensor_copy(packed[:, 1:2], negmin[:])
                nc.vector.tensor_copy(packed[:, 2:3], acc_max[:])
                nc.vector.tensor_copy(packed[0:1, 3:4], rtot[:])
                tps = rpsum.tile([4, 128], f32, tag="tps")
                nc.tensor.transpose(tps[:], packed[:], ident[:])
                tp = sp.tile([4, 128], f32, tag="tp")
                nc.scalar.activation(tp[:], tps[:], AF.Copy)
                cc_in = dram.tile([4, 128], f32, tag="ccin")
                cc_out = dram.tile([n_cores * 4, 128], f32, tag="ccout")
                nc.sync.dma_start(cc_in[:], tp[:])
                gt = sp.tile([1, n_cores * 512], f32, tag="gt")
                if NO_CC:
                    nc.vector.memset(gt[:], 0.0)
                    for j in range(4):
                        nc.sync.dma_start(
                            gt[0:1, 128 * j : 128 * (j + 1)], tp[j : j + 1, :]
                        )
                else:
                    nc.gpsimd.collective_compute(
                        "AllGather",
                        OP.bypass,
                        replica_groups=[list(range(n_cores))],
                        ins=[cc_in.opt()],
                        outs=[cc_out.opt()],
                    )
                    for j in range(n_cores * 4):
                        nc.sync.dma_start(
                            gt[0:1, 128 * j : 128 * (j + 1)], cc_out[j : j + 1, :]
                        )
                gtv = gt[0:1, :].rearrange("a (c f) -> a c f", c=n_cores)
                # global scalars, all on partition 0
                sS = sp.tile([1, 1], f32, tag="sS")    # S_nr
                sR = sp.tile([1, 1], f32, tag="sR")    # S_rw
                sNm = sp.tile([1, 1], f32, tag="sNm")  # -gmin
                sMx = sp.tile([1, 1], f32, tag="sMx")  # gmax
                nc.vector.tensor_reduce(
                    sS[:], gtv[:, :, 0:128], axis=mybir.AxisListType.XY, op=OP.add
                )
                nc.vector.tensor_reduce(
                    sNm[:], gtv[:, :, 128:256], axis=mybir.AxisListType.XY, op=OP.max
                )
                nc.vector.tensor_reduce(
                    sMx[:], gtv[:, :, 256:384], axis=mybir.AxisListType.XY, op=OP.max
                )
                nc.vector.tensor_reduce(
                    sR[:], gtv[:, :, 384:385], axis=mybir.AxisListType.XY, op=OP.add
                )
                c = sp.tile([1, 16], f32, tag="cw")
                # c0 = 1/S_nr ; c1 = f = S_rw/S_nr ; c2 = f25 = f/25
                nc.vector.reciprocal(c[:, 0:1], sS[:])
                nc.vector.tensor_tensor(c[:, 1:2], sR[:], c[:, 0:1], op=OP.mult)
                nc.vector.tensor_scalar(c[:, 2:3], c[:, 1:2], 1.0 / 25.0, None, OP.mult)
                # c3 = -gmin*f25 ; c4 = gmin*f25 ; c5 = mmin = clip(c4,0,1)
                nc.vector.tensor_tensor(c[:, 3:4], sNm[:], c[:, 2:3], op=OP.mult)
                nc.vector.tensor_scalar(c[:, 4:5], c[:, 3:4], -1.0, None, OP.mult)
                nc.vector.tensor_scalar(c[:, 5:6], c[:, 4:5], 0.0, 1.0, OP.max, OP.min)
                # c6 = gmax*f25 ; c7 = mmax
                nc.vector.tensor_tensor(c[:, 6:7], sMx[:], c[:, 2:3], op=OP.mult)
                nc.vector.tensor_scalar(c[:, 7:8], c[:, 6:7], 0.0, 1.0, OP.max, OP.min)
                # c8 = mmax - mmin ; c9 = gmax - gmin = sMx + sNm
                nc.vector.tensor_tensor(c[:, 8:9], c[:, 7:8], c[:, 5:6], op=OP.subtract)
                nc.vector.tensor_tensor(c[:, 9:10], sMx[:], sNm[:], op=OP.add)
                # c10 = 1/(gmax-gmin) ; c11 = inv25A
                nc.vector.reciprocal(c[:, 10:11], c[:, 9:10])
                nc.vector.tensor_tensor(c[:, 11:12], c[:, 8:9], c[:, 10:11], op=OP.mult)
                # c12 = -gmin*inv25A ; c13 = negBoverA = mmin - gmin*inv25A
                nc.vector.tensor_tensor(c[:, 12:13], sNm[:], c[:, 11:12], op=OP.mult)
                nc.vector.tensor_tensor(c[:, 13:14], c[:, 5:6], c[:, 12:13], op=OP.add)
                # c14 = thr = 7.5*inv25A ; c15 = -thr
                nc.vector.tensor_scalar(c[:, 14:15], c[:, 11:12], 7.5, None, OP.mult)
                nc.vector.tensor_scalar(c[:, 15:16], c[:, 14:15], -1.0, None, OP.mult)
                # pack [f25, inv25A, negBoverA, thr, negthr], broadcast via matmul
                b1 = sp.tile([1, 5], f32, tag="b1")
                nc.vector.tensor_copy(b1[:, 0:1], c[:, 2:3])
                nc.vector.tensor_copy(b1[:, 1:2], c[:, 11:12])
                nc.vector.tensor_copy(b1[:, 2:3], c[:, 13:14])
                nc.vector.tensor_copy(b1[:, 3:4], c[:, 14:15])
                nc.vector.tensor_copy(b1[:, 4:5], c[:, 15:16])
                bc_ps = rpsum.tile([128, 5], f32, tag="bcps")
                nc.tensor.matmul(bc_ps[:], lhsT=ones[:], rhs=b1[:])
                nc.scalar.activation(bcast[:], bc_ps[:], AF.Copy)

        # ---------------- Phase B: mask + morphology ----------------
        with tc.tile_pool(name="braw", bufs=10) as braw, \
             tc.tile_pool(name="bbf", bufs=8) as bbf, \
             tc.tile_pool(name="bzg", bufs=4) as bzg, \
             tc.tile_pool(name="bd", bufs=4) as bdp, \
             tc.tile_pool(name="bm", bufs=3) as bm, \
             tc.tile_pool(name="bout", bufs=3) as bout, \
             tc.tile_pool(name="bps", bufs=2, space="PSUM") as bps, \
             tc.tile_pool(name="mps", bufs=2, space="PSUM") as mps:

            scale_f = bcast[0:124, 0:1]
            scale_g = bcast[0:124, 1:2]
            bias_g = bcast[0:124, 2:3]
            thr = bcast[0:124, 3:4]

            for img in range(B_LOC):
                for b in range(B_BLOCKS):
                    r0 = B_STRIDE * b
                    g0 = r0 - 12
                    lo, hi = max(0, g0), min(H, g0 + 128)
                    p0, p1 = lo - g0, hi - g0
                    vout = min(B_STRIDE, H - r0)

                    bfs = {}
                    for ch in range(C):
                        for key, src in (("n", nr_ap), ("r", rf_ap)):
                            raw = braw.tile([128, W], f32, tag="raw")
                            if p0 > 0:
                                nc.scalar.memzero(raw[0:p0, :])
                            if p1 < 128:
                                _memset_rows(
                                    nc.vector.memset, lambda s, e: raw[s:e, :],
                                    p1, 128, 0.0,
                                )
                            nc.sync.dma_start(raw[p0:p1, :], src[img, ch, lo:hi, :])
                            bft = bbf.tile([128, W + 4], bf16, tag="bf")
                            nc.vector.memset(bft[:, 0:2], 0.0)
                            nc.vector.memset(bft[:, W + 2 : W + 4], 0.0)
                            if key == "n":
                                nc.scalar.activation(bft[:, 2 : W + 2], raw[:], AF.Copy)
                            else:
                                nc.vector.tensor_copy(bft[:, 2 : W + 2], raw[:])
                            bfs[(key, ch)] = bft

                    ds = []
                    for ch in range(C):
                        psn = bps.tile([124, W], f32, tag="bps")
                        for si, d in enumerate((-2, -1, 0, 1, 2)):
                            for c0 in (0, 512):
                                nc.tensor.matmul(
                                    psn[:, c0 : c0 + 512],
                                    lhsT=bb[:],
                                    rhs=bfs[("n", ch)][:, 2 + d + c0 : 2 + d + c0 + 512],
                                    start=si == 0, stop=si == 4,
                                )
                        psv = bps.tile([124, W], f32, tag="bps")
                        for si, d in enumerate((-2, -1, 0, 1, 2)):
                            for c0 in (0, 512):
                                nc.tensor.matmul(
                                    psv[:, c0 : c0 + 512],
                                    lhsT=bb[:],
                                    rhs=bfs[("r", ch)][:, 2 + d + c0 : 2 + d + c0 + 512],
                                    start=si == 0, stop=si == 4,
                                )
                        z = bzg.tile([124, W], bf16, tag="z")
                        nc.scalar.activation(z[:], psn[:], AF.Relu, scale=scale_f)
                        g = bzg.tile([124, W], bf16, tag="g")
                        nc.scalar.activation(
                            g[:], psv[:], AF.Identity, bias=bias_g, scale=scale_g
                        )
                        dt_ = bdp.tile([124, W], bf16, tag="d")
                        nc.vector.scalar_tensor_tensor(
                            dt_[:], z[:], 1.0, g[:], op0=OP.min, op1=OP.subtract
                        )
                        ds.append(dt_)
                    dM = bdp.tile([124, W], bf16, tag="dM")
                    nc.vector.tensor_tensor(dM[:], ds[0][:], ds[1][:], op=OP.max)
                    nc.vector.tensor_tensor(dM[:], dM[:], ds[2][:], op=OP.max)
                    dm = bdp.tile([124, W], bf16, tag="dm")
                    nc.vector.tensor_tensor(dm[:], ds[0][:], ds[1][:], op=OP.min)
                    nc.vector.tensor_tensor(dm[:], dm[:], ds[2][:], op=OP.min)
                    # q = (min_c d < -thr), bf16 0/1
                    q = bdp.tile([124, W], bf16, tag="q")
                    nc.vector.tensor_scalar(
                        q[:], dm[:], bcast[0:124, 4:5], None, OP.is_lt
                    )

                    # mask rows [0:124] ~ global rows r0-10 .. r0+113
                    # col-padded by 5 each side with erode-neutral 1.0
                    mask = bm.tile([128, W + 10], bf16, tag="mask")
                    nc.vector.memset(mask[0:124, 0:5], 1.0)
                    nc.vector.memset(mask[0:124, W + 5 : W + 10], 1.0)
                    m_hi = 124 if b < B_BLOCKS - 1 else 98
                    if b == B_BLOCKS - 1:
                        _memset_rows(
                            nc.vector.memset, lambda s, e: mask[s:e, :], 98, 124, 1.0
                        )
                    # mask = (max_c d > thr) | (min_c d < -thr)
                    nc.vector.scalar_tensor_tensor(
                        mask[0:m_hi, 5 : W + 5], dM[0:m_hi, :],
                        bcast[0:m_hi, 3:4], q[0:m_hi, :],
                        op0=OP.is_gt, op1=OP.logical_or,
                    )
                    if b == 0:
                        nc.vector.memset(mask[0:10, :], 1.0)
                    if debug:
                        nc.sync.dma_start(
                            dbg_ap[img, b, :, :], mask[0:124, 5 : W + 5]
                        )

                    # erode: horizontal min-tree (11 wide) + vertical 11-sum
                    t2 = bm.tile([128, W + 9], bf16, tag="t2")
                    nc.vector.tensor_tensor(
                        t2[0:124, :], mask[0:124, 0 : W + 9], mask[0:124, 1 : W + 10],
                        op=OP.min,
                    )
                    t4 = bm.tile([128, W + 7], bf16, tag="t4")
                    nc.vector.tensor_tensor(
                        t4[0:124, :], t2[0:124, 0 : W + 7], t2[0:124, 2 : W + 9],
                        op=OP.min,
                    )
                    t8 = bm.tile([128, W + 3], bf16, tag="t8")
                    nc.vector.tensor_tensor(
                        t8[0:124, :], t4[0:124, 0 : W + 3], t4[0:124, 4 : W + 7],
                        op=OP.min,
                    )
                    s11 = bm.tile([128, W], bf16, tag="s11")
                    nc.vector.tensor_tensor(
                        s11[0:124, :], t8[0:124, 0:W], t8[0:124, 3 : W + 3], op=OP.min
                    )
                    pse = mps.tile([114, W], f32, tag="mps")
                    for c0 in (0, 512):
                        nc.tensor.matmul(
                            pse[:, c0 : c0 + 512], lhsT=be[0:124, :],
                            rhs=s11[0:124, c0 : c0 + 512],
                        )
                    # eroded rows [0:114] ~ global rows r0-5 .. r0+108
                    er = bm.tile([128, W + 10], bf16, tag="er")
                    nc.vector.memset(er[0:114, 0:5], 0.0)
                    nc.vector.memset(er[0:114, W + 5 : W + 10], 0.0)
                    e_hi = 114 if b < B_BLOCKS - 1 else 93
                    if b == B_BLOCKS - 1:
                        _memset_rows(
                            nc.vector.memset, lambda s, e: er[s:e, :], 93, 114, 0.0
                        )
                    nc.scalar.activation(
                        er[0:e_hi, 5 : W + 5], pse[0:e_hi, :], AF.Relu,
                        bias=cm10[0:e_hi, :],
                    )
                    if b == 0:
                        nc.vector.memset(er[0:5, :], 0.0)

                    # dilate: horizontal max-tree + vertical 11-sum
                    u2 = bm.tile([128, W + 9], bf16, tag="u2")
                    nc.vector.tensor_tensor(
                        u2[0:114, :], er[0:114, 0 : W + 9], er[0:114, 1 : W + 10],
                        op=OP.max,
                    )
                    u4 = bm.tile([128, W + 7], bf16, tag="u4")
                    nc.vector.tensor_tensor(
                        u4[0:114, :], u2[0:114, 0 : W + 7], u2[0:114, 2 : W + 9],
                        op=OP.max,
                    )
                    u8 = bm.tile([128, W + 3], bf16, tag="u8")
                    nc.vector.tensor_tensor(
                        u8[0:114, :], u4[0:114, 0 : W + 3], u4[0:114, 4 : W + 7],
                        op=OP.max,
                    )
                    s11d = bm.tile([128, W], bf16, tag="s11d")
                    nc.vector.tensor_tensor(
                        s11d[0:114, :], u8[0:114, 0:W], u8[0:114, 3 : W + 3], op=OP.max
                    )
                    psd = mps.tile([104, W], f32, tag="mps")
                    for c0 in (0, 512):
                        nc.tensor.matmul(
                            psd[:, c0 : c0 + 512], lhsT=bd[0:114, :],
                            rhs=s11d[0:114, c0 : c0 + 512],
                        )
                    ng = bout.tile([104, W], f32, tag="ng")
                    nc.scalar.activation(
                        ng[:], psd[:], AF.Relu, bias=cp1[0:104, :], scale=-1.0
                    )
                    gh = bout.tile([104, W], f32, tag="gh")
                    nc.scalar.activation(
                        gh[:], ng[:], AF.Identity, bias=cp1[0:104, :], scale=-1.0
                    )
                    for ch in range(C):
                        nc.sync.dma_start(
                            gh_ap[img, ch, r0 : r0 + vout, :], gh[0:vout, :]
                        )
                        nc.sync.dma_start(
                            ng_ap[img, ch, r0 : r0 + vout, :], ng[0:vout, :]
                        )


_CACHE = {}


def build(n_cores=N_CORES, debug=False):
    key = (n_cores, debug)
    if key in _CACHE:
        return _CACHE[key]
    nc = bacc.Bacc(
        "TRN2",
        target_bir_lowering=False,
        debug=False,
        num_devices=n_cores,
    )
    with tile.TileContext(nc) as tc:
        build_body(nc, tc, n_cores, debug=debug)
    nc.compile()
    _CACHE[key] = nc
    return nc


def kernel(non_refer, refer, _trace=False):
    non_refer = np.ascontiguousarray(np.asarray(non_refer, np.float32))
    refer = np.ascontiguousarray(np.asarray(refer, np.float32))
    nc = build(N_CORES)
    consts = make_const_inputs()
    in_maps = [
        {
            "non_refer": non_refer[c * B_LOC : (c + 1) * B_LOC],
            "refer": refer[c * B_LOC : (c + 1) * B_LOC],
            **consts,
        }
        for c in range(N_CORES)
    ]
    res = run_bass_kernel_spmd(
        nc, in_maps, core_ids=list(range(N_CORES)), trace=_trace
    )
    ghost = np.concatenate([r["ghost"] for r in res.results], axis=0)
    nonghost = np.concatenate([r["nonghost"] for r in res.results], axis=0)
    kernel.last_results = res
    return ghost, nonghost


kernel.last_results = None
